# Initial kernel scaffold
#
"""GAT head (masked row-softmax attention + aggregation) on 8 TRN2 NeuronCores.

Sharding: rows of the NxN attention matrix are split across 8 cores (1024
each); w/a are replicated. Every core computes all h rows locally (redundant
compute on the idle PE beats the ~100us mesh-collective latency).

Attention tiles are held transposed [j, i] (j = neighbour on partitions,
i = own rows on the free dim) so the aggregation contracts over j on the PE,
with a ones-column of h_aug producing the softmax row-sums for free.

Row-uniform shift: logits are (leaky(x) - C_i)/32 with C_i = leaky(s_src_i+M)
>= row max (M = own-block max of s_dst). ACT Exp(scale=32) recovers
e^(l - C); the shift cancels in the row normalization. Per 128-neighbour
chunk one fused custom-DVE op (GAT_PRELU_SH) computes the shifted
leaky-relu logits into a quad buffer, an fp8 accum-DMA adds the {0,-1792}
mask, and one ACT Exp per 4-chunk quad yields bf16 weights for the PE.

h_aug tiles [j, 66] are produced directly by per-chunk PE matmuls with the
input chunk as the stationary operand (no transposes); col 64 carries
s_dst/32 (a_dst pre-scaled on host), col 65 is a ones column memset once.
"""
import os

import numpy as np
import ml_dtypes

N_NODES = 8192
D_IN = 512
F_OUT = 64
N_CORES = 8
R = N_NODES // N_CORES          # 1024 attention rows per core
NCHUNK = N_NODES // 128         # 64 j-chunks
DCHUNK = D_IN // 128            # 4 contraction chunks
SH = 32.0                       # logit-domain scale (Exp scale)

LAST_EXEC_NS = None
_CACHE = {}

MASK_BYTE = int(np.float32(-57344.0 / SH).astype(ml_dtypes.float8_e5m2)
                .view(np.uint8))


def _register_op1():
    from concourse.dve_spec import (AluOp, Bin, Spec, Src0, Src1, Zero, C0,
                                    C1, lower, maxx, _has_src1)
    from concourse.dve_uop import DveOpSpec
    import concourse.dve_ops as dve_ops

    if "GAT_PRELU_SH" in dve_ops._SUB_OPCODE_FOR_NAME:
        for op in dve_ops.OPS:
            if op.name == "GAT_PRELU_SH":
                return op

    # out = (x - 0.8*min(x,0)) + Src1 with x = Src0 + C0; C1 = -0.8 via s1.
    # (-0.8*min(x,0) == max(-0.8*x, 0))
    x = Bin(AluOp.ADD, Src0, C0)
    w = Bin(AluOp.MULTIPLY, x, C1)
    m = maxx(w, Zero)
    body = Bin(AluOp.ADD, Bin(AluOp.ADD, m, x), Src1)

    def ref(in0, in1, s0, s1, imm2):
        xx = in0.astype(np.float32) + s0
        return ((xx + np.maximum(xx * s1, 0)) + in1).astype(np.float32)

    spec = Spec(body=body, reference=ref)
    row = 1 + len(dve_ops.OPS)
    uops = lower(spec, ver="v3")
    sha = DveOpSpec(name="GAT_PRELU_SH", opcode=row, uops=uops,
                    rd1_en=_has_src1(spec)).sha("v3")
    op = dve_ops.DveOp("GAT_PRELU_SH", spec, subdim=False,
                       uops_sha={"v3": sha})
    dve_ops.OPS.append(op)
    dve_ops._SUB_OPCODE_FOR_NAME[op.name] = row
    dve_ops.CUSTOM_DVE_SPECS[op.name] = spec
    return op


def _build():
    import concourse.bacc as bacc
    import concourse.mybir as mybir
    import concourse.tile as tile
    from concourse.masks import make_identity

    op1 = _register_op1()

    F32 = mybir.dt.float32
    F32R = mybir.dt.float32r
    BF16 = mybir.dt.bfloat16
    FP16 = mybir.dt.float16
    FP8E5 = mybir.dt.float8e5
    AF = mybir.ActivationFunctionType
    OP = mybir.AluOpType
    FA = F_OUT + 2              # h_aug width: h(64) | sd32 | ones

    nc = bacc.Bacc("TRN2", target_bir_lowering=False, debug=False,
                   num_devices=N_CORES)

    inpre = nc.dram_tensor("inpre", [D_IN, N_NODES], FP16,
                           kind="ExternalInput")
    myinT = nc.dram_tensor("myinT", [D_IN, R], FP16, kind="ExternalInput")
    maskf8 = nc.dram_tensor("maskf8", [N_NODES, R], FP8E5,
                            kind="ExternalInput")
    w = nc.dram_tensor("w", [D_IN, F_OUT], FP16, kind="ExternalInput")
    wT = nc.dram_tensor("wT", [F_OUT, D_IN], F32, kind="ExternalInput")
    a2 = nc.dram_tensor("a2", [F_OUT, 2], F32, kind="ExternalInput")
    outd = nc.dram_tensor("out", [R, F_OUT], F32, kind="ExternalOutput")

    with tile.TileContext(nc) as tc:
        with tc.tile_pool(name="const", bufs=1) as const, \
             tc.tile_pool(name="haug", bufs=1) as haug, \
             tc.tile_pool(name="inp", bufs=1) as inp, \
             tc.tile_pool(name="hts", bufs=1) as hts, \
             tc.tile_pool(name="tq", bufs=3) as tq, \
             tc.tile_pool(name="qq", bufs=2) as qq, \
             tc.tile_pool(name="ep", bufs=2) as ep, \
             tc.tile_pool(name="psU", bufs=1, space="PSUM") as psU, \
             tc.tile_pool(name="psHd", bufs=2, space="PSUM") as psHd, \
             tc.tile_pool(name="psH", bufs=1, space="PSUM") as psH, \
             tc.tile_pool(name="psT", bufs=2, space="PSUM") as psT, \
             tc.tile_pool(name="psS", bufs=1, space="PSUM") as psS:

            # ---- constants -------------------------------------------------
            ident = const.tile([128, 128], F32, tag="ident")
            make_identity(nc, ident)
            ones_r = const.tile([1, 128], F32, tag="ones_r")
            nc.vector.memset(ones_r, 1.0)
            alpha = const.tile([128, 1], F32, tag="alpha")
            nc.vector.memset(alpha, 0.2)
            ones_c = const.tile([128, 1], F32, tag="ones_c")
            nc.vector.memset(ones_c, 1.0)
            wT_sb = const.tile([F_OUT, D_IN], F32, tag="wT_sb")
            nc.sync.dma_start(out=wT_sb, in_=wT[:, :])
            a2_sb = const.tile([F_OUT, 2], F32, tag="a2_sb")
            nc.sync.dma_start(out=a2_sb, in_=a2[:, :])

            # own-row input tiles (for the s_src row only)
            my_tiles = []
            for t2 in range(R // 512):
                mts = []
                for dc in range(DCHUNK):
                    mi = const.tile([128, 512], FP16, tag=f"minp{t2}_{dc}",
                                    name=f"minp{t2}_{dc}")
                    nc.sync.dma_start(
                        out=mi, in_=myinT[dc * 128:(dc + 1) * 128,
                                          t2 * 512:(t2 + 1) * 512])
                    mts.append(mi)
                my_tiles.append(mts)

            # ---- wa = w @ [a_src | a_dst/32] -------------------------------
            w_aug = []
            for dc in range(DCHUNK):
                pwa = psS.tile([128, 2], F32, tag="small")
                nc.tensor.matmul(pwa, wT_sb[:, dc * 128:(dc + 1) * 128], a2_sb,
                                 start=True, stop=True)
                wa = const.tile([128, FA], FP16, tag=f"waug{dc}",
                                name=f"waug{dc}")
                nc.sync.dma_start(out=wa[:, 0:F_OUT],
                                  in_=w[dc * 128:(dc + 1) * 128, :])
                nc.vector.tensor_copy(wa[:, F_OUT:F_OUT + 1], pwa[:, 1:2])
                w_aug.append(wa)

            # ---- persistent per-chunk tiles --------------------------------
            h_aug = [haug.tile([128, FA], BF16, tag=f"h{c}", name=f"h{c}")
                     for c in range(NCHUNK)]
            sd_lo = [const.tile([128, 1], F32, tag=f"sd{c}", name=f"sd{c}")
                     for c in range(NCHUNK)]
            u_ps = [psU.tile([FA, 512], F32, tag=f"u{h2}", name=f"u{h2}")
                    for h2 in range(2)]

            # ---- own h slabs -> s_row, M ----------------------------------
            s_row = const.tile([1, R], F32, tag="s_row")
            sdrow = [const.tile([1, 512], F32, tag=f"sdrow{t2}",
                                name=f"sdrow{t2}") for t2 in range(2)]
            for t2 in range(R // 512):
                phm = psH.tile([F_OUT + 1, 512], F32, tag="hT")
                for dc in range(DCHUNK):
                    nc.tensor.matmul(phm, w_aug[dc][:, 0:F_OUT + 1],
                                     my_tiles[t2][dc],
                                     start=(dc == 0), stop=(dc == DCHUNK - 1))
                hTm = hts.tile([F_OUT + 1, 512], F32, tag="hT_sb")
                nc.vector.tensor_copy(hTm, phm)
                pss = psS.tile([1, 512], F32, tag="small")
                nc.tensor.matmul(pss, a2_sb[:, 0:1], hTm[0:F_OUT, :],
                                 start=True, stop=True)
                nc.vector.tensor_copy(s_row[0:1, t2 * 512:(t2 + 1) * 512],
                                      pss[0:1, :])
                nc.sync.dma_start(out=sdrow[t2],
                                  in_=hTm[F_OUT:F_OUT + 1, :])

            # ---- M = max own sd32 (x32); C row; broadcasts ----------------
            sdmax = const.tile([1, 512], F32, tag="sdmax")
            nc.vector.tensor_tensor(sdmax, sdrow[0], sdrow[1], OP.max)
            mred = const.tile([1, 1], F32, tag="mred")
            nc.vector.tensor_reduce(mred, sdmax, mybir.AxisListType.X, OP.max)
            m_true = const.tile([1, 1], F32, tag="m_true")
            nc.vector.tensor_scalar(m_true, mred, float(SH), None, OP.mult)

            c_row = const.tile([1, R], F32, tag="c_row")
            nc.scalar.activation(c_row, s_row, AF.Prelu,
                                 bias=m_true[0:1, 0:1], scale=1.0,
                                 alpha=alpha[0:1, 0:1])
            negc32 = const.tile([1, R], F32, tag="negc32")
            nc.vector.tensor_scalar(negc32, c_row, -1.0 / SH, None, OP.mult)
            s_row32 = const.tile([1, R], F32, tag="s_row32")
            nc.vector.tensor_scalar(s_row32, s_row, 1.0 / SH, None, OP.mult)

            bc_src32 = const.tile([128, R], F32, tag="bc_src32")
            bc_c32n = const.tile([128, R], F32, tag="bc_c32n")
            for t2 in range(R // 512):
                seg = slice(t2 * 512, (t2 + 1) * 512)
                pbc = psS.tile([128, 512], F32, tag="small")
                nc.tensor.matmul(pbc, ones_r, s_row32[0:1, seg],
                                 start=True, stop=True)
                nc.vector.tensor_copy(bc_src32[:, seg], pbc)
                pbc2 = psS.tile([128, 512], F32, tag="small")
                nc.tensor.matmul(pbc2, ones_r, negc32[0:1, seg],
                                 start=True, stop=True)
                nc.vector.tensor_copy(bc_c32n[:, seg], pbc2)

            for c in range(NCHUNK):
                nc.vector.tensor_copy(h_aug[c][:, F_OUT + 1:FA], ones_c)

            # ---- per-chunk h_aug via direct PE matmuls --------------------
            def h_quad(t):
                its = []
                for dc in range(DCHUNK):
                    it = inp.tile([128, 512], FP16, tag=f"inp{dc}", bufs=2)
                    nc.sync.dma_start(
                        out=it, in_=inpre[dc * 128:(dc + 1) * 128,
                                          t * 512:(t + 1) * 512])
                    its.append(it)
                for jj in range(4):
                    c = 4 * t + jj
                    ph = psHd.tile([128, FA], F32, tag="hd")
                    for dc in range(DCHUNK):
                        nc.tensor.matmul(
                            ph[:, 0:F_OUT + 1],
                            its[dc][:, jj * 128:(jj + 1) * 128],
                            w_aug[dc][:, 0:F_OUT + 1],
                            start=(dc == 0), stop=(dc == DCHUNK - 1))
                    nc.scalar.activation(h_aug[c][:, 0:F_OUT + 1],
                                         ph[:, 0:F_OUT + 1], AF.Copy,
                                         bias=0.0, scale=1.0)
                    nc.vector.tensor_copy(sd_lo[c], ph[:, F_OUT:F_OUT + 1])

            # ---- main quad loop -------------------------------------------
            emit_idx = [0]

            def quad(chunks):
                tquad = tq.tile([128, 4096], F32, tag="t")
                for k, c in enumerate(chunks):
                    slot = tquad[:, k * 1024:(k + 1) * 1024]
                    nc.vector._custom_dve(op1, out=slot, in0=bc_src32,
                                          in1=bc_c32n,
                                          s0=sd_lo[c][:, 0:1], s1=-0.8)
                    nc.gpsimd.dma_start(
                        out=slot, in_=maskf8[c * 128:(c + 1) * 128, :],
                        accum_op=OP.add)
                qquad = qq.tile([128, 4096], BF16, tag="q")
                nc.scalar.activation(qquad, tquad, AF.Exp, bias=0.0,
                                     scale=float(SH))
                first = emit_idx[0] == 0
                last = emit_idx[0] == NCHUNK - 4
                emit_idx[0] += 4
                for h2 in range(2):
                    for k, c in enumerate(chunks):
                        nc.tensor.matmul(
                            u_ps[h2], h_aug[c],
                            qquad[:, k * 1024 + h2 * 512:
                                  k * 1024 + (h2 + 1) * 512],
                            start=(first and k == 0),
                            stop=(last and k == 3))

            for t in range(3):
                h_quad(t)
            for qi in range(NCHUNK // 4):
                if qi + 3 < NCHUNK // 4:
                    h_quad(qi + 3)
                quad(list(range(4 * qi, 4 * qi + 4)))

            # ---- epilogue: transpose u.T, normalize, ELU ------------------
            uT_sb = const.tile([FA, R], F32, tag="uT_sb")
            for h2 in range(2):
                nc.vector.tensor_copy(uT_sb[:, h2 * 512:(h2 + 1) * 512],
                                      u_ps[h2])
            for k2 in range(R // 128):
                tr2 = psT.tile([128, FA], F32, tag="tr")
                nc.tensor.transpose(
                    tr2[:, 0:FA],
                    uT_sb[:, k2 * 128:(k2 + 1) * 128],
                    ident[0:FA, 0:FA])
                rc = ep.tile([128, 1], F32, tag="rc")
                nc.vector.reciprocal(rc, tr2[:, F_OUT + 1:FA])
                xs = ep.tile([128, F_OUT], F32, tag="xs")
                nc.vector.tensor_scalar(xs, tr2[:, 0:F_OUT], rc[:, 0:1], None,
                                        OP.mult)
                cm = ep.tile([128, F_OUT], F32, tag="cm")
                nc.vector.tensor_scalar(cm, xs, 0.0, None, OP.min)
                ex = ep.tile([128, F_OUT], F32, tag="ex")
                nc.scalar.activation(ex, cm, AF.Exp, bias=0.0, scale=1.0)
                em = ep.tile([128, F_OUT], F32, tag="em")
                nc.vector.tensor_scalar(em, ex, -1.0, None, OP.add)
                ot = ep.tile([128, F_OUT], F32, tag="ot")
                nc.vector.tensor_tensor(ot, xs, em, OP.max)
                nc.sync.dma_start(out=outd[k2 * 128:(k2 + 1) * 128, :],
                                  in_=ot)

    nc.compile()
    return nc


def kernel(input, adj, w, a):
    global LAST_EXEC_NS
    from concourse.bass_utils import run_bass_kernel_spmd

    if "nc" not in _CACHE:
        _CACHE["nc"] = _build()
    nc = _CACHE["nc"]

    input = np.asarray(input, dtype=np.float32)
    adj = np.asarray(adj)
    w = np.asarray(w, dtype=np.float32)
    a = np.asarray(a, dtype=np.float32).reshape(2 * F_OUT)

    inputT = np.ascontiguousarray(input.T)                      # [512, 8192]
    inputT16 = inputT.astype(np.float16)
    wT = np.ascontiguousarray(w.T)                              # [64, 512]
    w16 = w.astype(np.float16)
    a2 = np.ascontiguousarray(
        np.stack([a[:F_OUT], a[F_OUT:] / SH], axis=1))          # [64, 2]

    in_maps = []
    for k in range(N_CORES):
        cols = slice(k * R, (k + 1) * R)
        adjc = np.ascontiguousarray(adj[:, cols])
        cmpl = (adjc == 0).astype(np.uint8)                     # complement
        mf8 = (cmpl * MASK_BYTE).view(ml_dtypes.float8_e5m2)    # {0,-1792}
        in_maps.append({
            "inpre": inputT16,
            "myinT": np.ascontiguousarray(inputT16[:, cols]),
            "maskf8": mf8,
            "w": w16,
            "wT": wT,
            "a2": a2,
        })

    trace = bool(os.environ.get("GAT_TRACE"))
    res = run_bass_kernel_spmd(nc, in_maps, list(range(N_CORES)), trace=trace)
    LAST_EXEC_NS = res.exec_time_ns
    return np.concatenate([res.results[k]["out"] for k in range(N_CORES)],
                          axis=0)



# revision 9
# speedup vs baseline: 1.7950x; 1.7950x over previous
"""GAT head (masked row-softmax attention + aggregation) on 8 TRN2 NeuronCores.

Sharding: rows of the NxN attention matrix are split across 8 cores (1024
each); w/a are replicated. Every core computes all h rows locally. The input
columns (and mask rows) are permuted per-core so the core's OWN 1024 nodes
come first — s_src falls out of the first two h-quads with no separate
input pass.

Attention tiles are held transposed [j, i] (j = neighbour on partitions,
i = own rows on the free dim) so the aggregation contracts over j on the PE;
a ones-column of h_aug produces the softmax row-sums for free.

No logit shift: logits stay unscaled (max ~70, exp fits bf16/f32 by miles).
Per 128-neighbour chunk one fused custom-DVE op computes
leaky(s_src_i + s_dst_j) + mask directly, reading the {0,-57344} fp8 mask
tile as the op's second input — no accumulate-DMA, no gpsimd involvement.
One ACT Exp per 4-chunk quad yields bf16 weights for the PE.

h tiles [j, 66] come from per-chunk PE matmuls with the input chunk as the
stationary operand; cols 64/65 carry s_dst/s_src (w pre-multiplied with a on
the PE). DMAs are batched per-quad ([512,1024] mask, [512,512] input slabs
via 3D access patterns) on the HWDGE sync queue.
"""
import os

import numpy as np
import ml_dtypes

N_NODES = 8192
D_IN = 512
F_OUT = 64
N_CORES = 8
R = N_NODES // N_CORES          # 1024 attention rows per core
NCHUNK = N_NODES // 128         # 64 j-chunks
NQUAD = NCHUNK // 4             # 16 quads
DCHUNK = D_IN // 128            # 4 contraction chunks

LAST_EXEC_NS = None
_CACHE = {}

MASK_VAL = -57344.0             # most-negative finite fp8e5m2
MASK_BYTE = int(np.float32(MASK_VAL).astype(ml_dtypes.float8_e5m2)
                .view(np.uint8))


def _find_op(name):
    import concourse.dve_ops as dve_ops
    if name in dve_ops._SUB_OPCODE_FOR_NAME:
        for op in dve_ops.OPS:
            if op.name == name:
                return op
    return None


def _add_op(name, spec):
    from concourse.dve_spec import lower, _has_src1
    from concourse.dve_uop import DveOpSpec
    import concourse.dve_ops as dve_ops

    row = 1 + len(dve_ops.OPS)
    uops = lower(spec, ver="v3")
    sha = DveOpSpec(name=name, opcode=row, uops=uops,
                    rd1_en=_has_src1(spec)).sha("v3")
    op = dve_ops.DveOp(name, spec, subdim=False, uops_sha={"v3": sha})
    dve_ops.OPS.append(op)
    dve_ops._SUB_OPCODE_FOR_NAME[op.name] = row
    dve_ops.CUSTOM_DVE_SPECS[op.name] = spec
    return op


def _register_gat():
    """out = leaky(in0 + s0) + in1  (s1 = -0.8 gives slope 0.2)."""
    from concourse.dve_spec import AluOp, Bin, Spec, Src0, Src1, Zero, C0, \
        C1, maxx

    op = _find_op("GAT_PRELU_SH")
    if op is not None:
        return op
    x = Bin(AluOp.ADD, Src0, C0)
    w = Bin(AluOp.MULTIPLY, x, C1)
    m = maxx(w, Zero)
    body = Bin(AluOp.ADD, Bin(AluOp.ADD, m, x), Src1)

    def ref(in0, in1, s0, s1, imm2):
        xx = in0.astype(np.float32) + s0
        return ((xx + np.maximum(xx * s1, 0))
                + in1.astype(np.float32)).astype(np.float32)

    return _add_op("GAT_PRELU_SH", Spec(body=body, reference=ref))


def _register_fin():
    """out = max(min(in1, s1) + s0, in0) — ELU tail with s0=-1, s1=1."""
    from concourse.dve_spec import AluOp, Bin, Spec, Src0, Src1, C0, C1, \
        maxx, minn

    op = _find_op("GAT_ELU_FIN")
    if op is not None:
        return op
    a = minn(Src1, C1)
    b = Bin(AluOp.ADD, a, C0)
    body = maxx(b, Src0)

    def ref(in0, in1, s0, s1, imm2):
        return np.maximum(np.minimum(in1.astype(np.float32), s1) + s0,
                          in0.astype(np.float32)).astype(np.float32)

    return _add_op("GAT_ELU_FIN", Spec(body=body, reference=ref))


def _build():
    import concourse.bacc as bacc
    import concourse.mybir as mybir
    import concourse.tile as tile
    from concourse.masks import make_identity

    op_gat = _register_gat()
    op_fin = _register_fin()

    F32 = mybir.dt.float32
    BF16 = mybir.dt.bfloat16
    FP16 = mybir.dt.float16
    FP8E5 = mybir.dt.float8e5
    AF = mybir.ActivationFunctionType
    OP = mybir.AluOpType

    nc = bacc.Bacc("TRN2", target_bir_lowering=False, debug=False,
                   num_devices=N_CORES)

    inpre = nc.dram_tensor("inpre", [D_IN, N_NODES], FP16,
                           kind="ExternalInput")
    maskf8 = nc.dram_tensor("maskf8", [N_NODES, R], FP8E5,
                            kind="ExternalInput")
    w = nc.dram_tensor("w", [D_IN, F_OUT], FP16, kind="ExternalInput")
    wT = nc.dram_tensor("wT", [F_OUT, D_IN], F32, kind="ExternalInput")
    a2 = nc.dram_tensor("a2", [F_OUT, 2], F32, kind="ExternalInput")
    outd = nc.dram_tensor("out", [R, F_OUT], F32, kind="ExternalOutput")

    def q3(ap, k):                      # [p, (k c)] -> [p, k, c]
        return ap.rearrange("p (k c) -> p k c", k=k)

    with tile.TileContext(nc) as tc:
        with tc.tile_pool(name="const", bufs=1) as const, \
             tc.tile_pool(name="inp", bufs=3) as inp, \
             tc.tile_pool(name="haug", bufs=6) as haug, \
             tc.tile_pool(name="sdp", bufs=6) as sdp, \
             tc.tile_pool(name="mqp", bufs=4) as mqp, \
             tc.tile_pool(name="tqp", bufs=3) as tqp, \
             tc.tile_pool(name="qqp", bufs=3) as qqp, \
             tc.tile_pool(name="ep", bufs=4) as ep, \
             tc.tile_pool(name="psU", bufs=1, space="PSUM") as psU, \
             tc.tile_pool(name="psH", bufs=2, space="PSUM") as psH, \
             tc.tile_pool(name="psT", bufs=2, space="PSUM") as psT, \
             tc.tile_pool(name="psS", bufs=2, space="PSUM") as psS:

            hq_of = {}
            sd_of = {}
            mq_of = {}

            def issue_inp(t):
                it = inp.tile([128, 4 * D_IN], FP16, tag="it")
                nc.sync.dma_start(
                    out=q3(it[:], 4),
                    in_=inpre[:, 512 * t:512 * (t + 1)]
                    .rearrange("(k p) c -> p k c", p=128))
                return it

            def issue_mask(q):
                mq = mqp.tile([128, 4096], FP8E5, tag="mq")
                nc.sync.dma_start(
                    out=q3(mq[:], 4),
                    in_=maskf8[512 * q:512 * (q + 1), :]
                    .rearrange("(k p) c -> p k c", p=128))
                mq_of[q] = mq

            # ---- front-load the latency-critical DMAs ----------------------
            wT_sb = const.tile([F_OUT, D_IN], F32, tag="wT_sb")
            nc.scalar.dma_start(out=wT_sb, in_=wT[:, :])
            a2_sb = const.tile([F_OUT, 2], F32, tag="a2_sb")
            nc.scalar.dma_start(out=a2_sb, in_=a2[:, :])
            it01 = [issue_inp(0), issue_inp(1)]
            issue_mask(0)

            ident = const.tile([128, 128], F32, tag="ident")
            make_identity(nc, ident)
            ones128 = const.tile([128, 128], F32, tag="ones128")
            nc.vector.memset(ones128, 1.0)

            # ---- w_aug = [w | w@a_dst]; wa_bc[d, :] = (w@a_src)[d] ---------
            w_aug = []
            wa_bc = []
            for dc in range(DCHUNK):
                pwa = psS.tile([128, 2], F32, tag="small")
                nc.tensor.matmul(pwa, wT_sb[:, dc * 128:(dc + 1) * 128],
                                 a2_sb, start=True, stop=True)
                wa = const.tile([128, 65], FP16, tag=f"waug{dc}",
                                name=f"waug{dc}")
                nc.scalar.dma_start(out=wa[:, 0:F_OUT],
                                    in_=w[dc * 128:(dc + 1) * 128, :])
                nc.vector.tensor_copy(wa[:, F_OUT:F_OUT + 1], pwa[:, 0:1])
                w_aug.append(wa)
                wb = const.tile([128, 128], FP16, tag=f"wabc{dc}",
                                name=f"wabc{dc}")
                nc.vector.tensor_scalar(wb, ones128, pwa[:, 1:2], None,
                                        OP.mult)
                wa_bc.append(wb)
            for q in range(1, 4):
                issue_mask(q)

            bc_src = const.tile([128, R], F32, tag="bc_src")
            u_ps = [psU.tile([F_OUT + 1, 512], F32, tag=f"u{h2}",
                             name=f"u{h2}") for h2 in range(2)]

            # bc_src[p, i] = s_src[i] directly off the input slabs
            for t in range(2):
                pbc = psS.tile([128, 512], F32, tag="small")
                for dc in range(DCHUNK):
                    nc.tensor.matmul(pbc, wa_bc[dc],
                                     it01[t][:, dc * 512:(dc + 1) * 512],
                                     start=(dc == 0), stop=(dc == DCHUNK - 1))
                nc.scalar.activation(bc_src[:, 512 * t:512 * (t + 1)], pbc,
                                     AF.Copy, bias=0.0, scale=1.0)

            # ---- per-quad h tiles ------------------------------------------
            def h_quad(t, it=None):
                if it is None:
                    it = issue_inp(t)
                ph = psH.tile([128, 4 * 65], F32, tag="ph")
                for jj in range(4):
                    for dc in range(DCHUNK):
                        nc.tensor.matmul(
                            ph[:, 65 * jj:65 * jj + 65],
                            it[:, dc * 512 + jj * 128:
                               dc * 512 + jj * 128 + 128],
                            w_aug[dc],
                            start=(dc == 0), stop=(dc == DCHUNK - 1))
                ph3 = q3(ph[:], 4)
                hq = haug.tile([128, 4 * 65], BF16, tag="hq")
                hq3 = q3(hq[:], 4)
                nc.scalar.activation(hq3[:, :, 0:F_OUT], ph3[:, :, 0:F_OUT],
                                     AF.Copy, bias=0.0, scale=1.0)
                nc.gpsimd.memset(hq3[:, :, F_OUT:F_OUT + 1], 1.0)
                sd4 = sdp.tile([128, 4], F32, tag="sd4")
                nc.scalar.activation(
                    sd4, ph3[:, :, F_OUT:F_OUT + 1]
                    .rearrange("p k c -> p (k c)"),
                    AF.Copy, bias=0.0, scale=1.0)
                hq_of[t] = hq
                sd_of[t] = sd4

            # ---- attention quad -------------------------------------------
            # split=True pipelines exp/agg per chunk — shorter fill/drain at
            # the ends of the run at the cost of 3 extra ACT dispatches.
            def quad(q, split=False):
                if q not in mq_of:
                    issue_mask(q)
                mq = mq_of.pop(q)
                tq = tqp.tile([128, 4096], F32, tag="tq")
                sd4 = sd_of.pop(q)
                hq = hq_of.pop(q)
                qq = qqp.tile([128, 4096], BF16, tag="qq")

                def aggs(k):
                    for h2 in range(2):
                        nc.tensor.matmul(
                            u_ps[h2], hq[:, 65 * k:65 * k + 65],
                            qq[:, 1024 * k + 512 * h2:
                               1024 * k + 512 * (h2 + 1)],
                            start=(q == 0 and k == 0),
                            stop=(q == NQUAD - 1 and k == 3))

                for k in range(4):
                    nc.vector._custom_dve(
                        op_gat, out=tq[:, 1024 * k:1024 * (k + 1)],
                        in0=bc_src, in1=mq[:, 1024 * k:1024 * (k + 1)],
                        s0=sd4[:, k:k + 1], s1=-0.8)
                    if split:
                        nc.scalar.activation(
                            qq[:, 1024 * k:1024 * (k + 1)],
                            tq[:, 1024 * k:1024 * (k + 1)],
                            AF.Exp, bias=0.0, scale=1.0)
                        aggs(k)
                if not split:
                    nc.scalar.activation(qq, tq, AF.Exp, bias=0.0, scale=1.0)
                    for k in range(4):
                        aggs(k)

            # ---- prologue h tiles ------------------------------------------
            h_quad(0, it=it01[0])
            h_quad(1, it=it01[1])
            h_quad(2)
            h_quad(3)

            # ---- main loop -------------------------------------------------
            for q in range(NQUAD):
                if q + 4 < NQUAD:
                    h_quad(q + 4)
                quad(q, split=(q == 0 or q == NQUAD - 1))

            # ---- epilogue: transpose u, normalize, ELU ---------------------
            uT_sb = const.tile([F_OUT + 1, R], F32, tag="uT_sb")
            for h2 in range(2):
                nc.scalar.activation(uT_sb[:, 512 * h2:512 * (h2 + 1)],
                                     u_ps[h2], AF.Copy, bias=0.0, scale=1.0)
            out_sb = const.tile([128, 8 * F_OUT], F32, tag="out_sb")
            for k2 in range(8):
                tr = psT.tile([128, F_OUT + 1], F32, tag="tr")
                nc.tensor.transpose(tr, uT_sb[:, 128 * k2:128 * (k2 + 1)],
                                    ident[0:F_OUT + 1, 0:F_OUT + 1])
                rc = ep.tile([128, 1], F32, tag="rc")
                nc.vector.reciprocal_approx_fast(rc, tr[:, F_OUT:F_OUT + 1])
                xs = ep.tile([128, F_OUT], F32, tag="xs")
                nc.scalar.activation(xs, tr[:, 0:F_OUT], AF.Copy, bias=0.0,
                                     scale=rc[:, 0:1])
                ex = ep.tile([128, F_OUT], F32, tag="ex")
                nc.scalar.activation(ex, tr[:, 0:F_OUT], AF.Exp, bias=0.0,
                                     scale=rc[:, 0:1])
                nc.vector._custom_dve(
                    op_fin, out=out_sb[:, F_OUT * k2:F_OUT * (k2 + 1)],
                    in0=xs, in1=ex, s0=-1.0, s1=1.0)
            nc.sync.dma_start(
                out=outd[:, :].rearrange("(k p) c -> p k c", p=128),
                in_=q3(out_sb[:], 8))

    nc.compile()
    return nc


def kernel(input, adj, w, a):
    global LAST_EXEC_NS
    from concourse.bass_utils import run_bass_kernel_spmd

    if "nc" not in _CACHE:
        _CACHE["nc"] = _build()
    nc = _CACHE["nc"]

    input = np.asarray(input, dtype=np.float32)
    adj = np.asarray(adj)
    w = np.asarray(w, dtype=np.float32)
    a = np.asarray(a, dtype=np.float32).reshape(2 * F_OUT)

    inputT16 = np.ascontiguousarray(input.T).astype(np.float16)  # [512, 8192]
    wT = np.ascontiguousarray(w.T)                               # [64, 512]
    w16 = w.astype(np.float16)
    a2 = np.ascontiguousarray(
        np.stack([a[F_OUT:], a[:F_OUT]], axis=1))                # [a_dst|a_src]

    in_maps = []
    allr = np.arange(N_NODES)
    for k in range(N_CORES):
        cols = slice(k * R, (k + 1) * R)
        perm = np.concatenate([allr[cols], allr[:k * R], allr[(k + 1) * R:]])
        adjc = adj[:, cols][perm]                     # [8192, 1024], perm rows
        mf8 = (((adjc == 0).astype(np.uint8)) * MASK_BYTE) \
            .view(ml_dtypes.float8_e5m2)
        in_maps.append({
            "inpre": np.ascontiguousarray(inputT16[:, perm]),
            "maskf8": np.ascontiguousarray(mf8),
            "w": w16,
            "wT": wT,
            "a2": a2,
        })

    trace = bool(os.environ.get("GAT_TRACE"))
    res = run_bass_kernel_spmd(nc, in_maps, list(range(N_CORES)), trace=trace)
    LAST_EXEC_NS = res.exec_time_ns
    return np.concatenate([res.results[k]["out"] for k in range(N_CORES)],
                          axis=0)
